# revision 1
# baseline (speedup 1.0000x reference)
"""Trainium2 Bass kernel for nn_BaseLinearSSM.

y[b,t] = Re(C @ x_{t+1}) + D @ u[b,t] + bias,  x_{t+1} = A x_t + B u_t  (complex A,B,C)

Strategy:
  Host (fp64): eigendecompose A = V diag(w) V^-1  (cond(V) ~ 370 for this
  problem class), fold V into B/C:  Bt = V^-1 B, Ct = C V.  The recurrence
  becomes diagonal:  xt_{t+1} = w * xt_t + Bt u_t.  Writing w = rho*e^{i th},
  z_t = e^{-i th t} xt_t obeys  z_t = rho * z_{t-1} + e^{-i th t} (Bt u)_t --
  two *real* first-order scans per mode, which map 1:1 onto the DVE's native
  tensor_tensor_scan (state = data0*state + data1).

  Device (per core, batch-sharded 2 of 16):
    f = Bt^T-matmuls of u  ->  modulate by cos/sin(th*t) tables (host fp64)
    -> tensor_tensor_scan along t  ->  demodulate  ->  y = CtRe.x_r - CtIm.x_i
    + D u accumulated in one PSUM group.

  Cores are fully independent (A/B/C/D replicated); host shards u and
  gathers y.
"""

import sys

import numpy as np

if "/opt/trn_rl_repo" not in sys.path:
    sys.path.insert(0, "/opt/trn_rl_repo")

BATCH, T, IN, OUT, N = 16, 2048, 128, 128, 512
NCORES = 8
BLOCAL = BATCH // NCORES  # 2
COLS = BLOCAL * T         # 4096 columns per core, col = b*T + t
NT = N // 128             # 4 partition tiles over the state dim
BLK = 512                 # columns per pipeline block
NBLK = COLS // BLK        # 8 blocks, (b, tb) with tb in 0..3
TBLK = T // BLK           # 4 t-blocks per batch element
# blob pieces (also DMA issue order):
#   p0: ut | btr | bti          (gates the f-matmuls)
#   p1: tb0 tables              (gates the first modulate)
#   p2: dwt | ctr | cti | rho   (gates y-projection / scans)
#   p3..p5: tb1..tb3 tables
P0W = COLS + N + N
TBW = 2 * NT * BLK  # one tb's cos+sin tables
P2W = OUT + NT * OUT + NT * OUT + NT * BLK
BLOBW = P0W + P2W + TBLK * TBW

LAST_RESULT = None  # BassKernelResults of the most recent run (for profiling)

_NC_CACHE = None


def _build_nc():
    """Build the SPMD Bass program (identical on all 8 cores)."""
    from concourse import bass, mybir
    from concourse import tile

    dt = mybir.dt.float32
    op = mybir.AluOpType

    nc = bass.Bass("TRN2", target_bir_lowering=False, debug=False)

    # All inputs packed in ONE [128, W] blob -> one DMA -> one HW queue ->
    # at most one DMA sync wait on any consumer (fused fp32 LDW+MATMUL
    # supports a single sync wait).
    blob = nc.dram_tensor("blob", [128, BLOBW], dt, kind="ExternalInput")
    yout = nc.dram_tensor("y", [OUT, COLS], dt, kind="ExternalOutput")  # [o, b*T+t]

    with tile.TileContext(nc) as tc:
        with (
            tc.tile_pool(name="const", bufs=1) as cpool,
            tc.tile_pool(name="tmp", bufs=2) as tpool,
            tc.tile_pool(name="gp", bufs=1) as gpool,
            tc.tile_pool(name="zp", bufs=2) as zpool,
            tc.tile_pool(name="xr", bufs=1) as xrpool,
            tc.tile_pool(name="xi", bufs=2) as xipool,
            tc.tile_pool(name="ysb", bufs=2) as spool,
            tc.tile_pool(name="fps", bufs=6, space="PSUM") as fpool,
            tc.tile_pool(name="yps", bufs=2, space="PSUM") as ypool,
        ):
            blob_sb = cpool.tile([128, BLOBW], dt)
            # Issue order = dependency order: f-matmul inputs, first tables,
            # projection weights, remaining tables.  _legalize_multi_waits
            # keeps any resulting wait pairing legal for walrus.
            bounds = [0, P0W, P0W + TBW, P0W + TBW + P2W]
            for k in range(2, TBLK + 1):
                bounds.append(bounds[-1] + TBW)
            for a, bnd in zip(bounds[:-1], bounds[1:]):
                nc.sync.dma_start(blob_sb[:, a:bnd], blob[:, a:bnd])
            o = [0]
            def take(w):
                s = blob_sb[:, o[0]:o[0] + w]
                o[0] += w
                return s
            ut_sb = take(COLS)
            btr_sb = take(N)
            bti_sb = take(N)
            ct_tb = [[None] * NT for _ in range(TBLK)]
            st_tb = [[None] * NT for _ in range(TBLK)]
            for m in range(NT):
                ct_tb[0][m] = take(BLK)
            for m in range(NT):
                st_tb[0][m] = take(BLK)
            dwt_sb = take(OUT)
            ctr_sb = take(NT * OUT)
            cti_sb = take(NT * OUT)
            rho_sb = [take(BLK) for _ in range(NT)]
            for k in range(1, TBLK):
                for m in range(NT):
                    ct_tb[k][m] = take(BLK)
                for m in range(NT):
                    st_tb[k][m] = take(BLK)
            assert o[0] == BLOBW

            zr_prev = [None] * NT
            zi_prev = [None] * NT
            for b in range(BLOCAL):
                for tb in range(TBLK):
                    col0 = b * T + tb * BLK
                    ucols = ut_sb[:, col0:col0 + BLK]
                    xr_blk = [None] * NT
                    xi_blk = [None] * NT
                    for m in range(NT):
                        ctt = ct_tb[tb][m][:]
                        stt = st_tb[tb][m][:]
                        # f = Bt u  (complex), PSUM
                        fre = fpool.tile([128, BLK], dt, tag="f")
                        fim = fpool.tile([128, BLK], dt, tag="f")
                        nc.tensor.matmul(
                            fre[:], btr_sb[:, m * 128:(m + 1) * 128], ucols
                        )
                        nc.tensor.matmul(
                            fim[:], bti_sb[:, m * 128:(m + 1) * 128], ucols
                        )
                        # modulate: g = e^{-i th t} f
                        t1 = tpool.tile([128, BLK], dt, tag="t1")
                        t2 = tpool.tile([128, BLK], dt, tag="t2")
                        nc.vector.tensor_tensor(t1[:], ctt, fre[:], op=op.mult)
                        nc.vector.tensor_tensor(t2[:], stt, fim[:], op=op.mult)
                        gr = gpool.tile([128, BLK], dt, tag=f"gr{m}")
                        nc.vector.tensor_tensor(gr[:], t1[:], t2[:], op=op.add)
                        t3 = tpool.tile([128, BLK], dt, tag="t1")
                        t4 = tpool.tile([128, BLK], dt, tag="t2")
                        nc.vector.tensor_tensor(t3[:], ctt, fim[:], op=op.mult)
                        nc.vector.tensor_tensor(t4[:], stt, fre[:], op=op.mult)
                        gi = gpool.tile([128, BLK], dt, tag=f"gi{m}")
                        nc.vector.tensor_tensor(gi[:], t3[:], t4[:], op=op.subtract)
                        # scan: z = rho*z_prev + g along t (chained across tb)
                        zr = zpool.tile([128, BLK], dt, tag=f"zr{m}")
                        zi = zpool.tile([128, BLK], dt, tag=f"zi{m}")
                        init_r = 0.0 if tb == 0 else zr_prev[m][:, BLK - 1:BLK]
                        init_i = 0.0 if tb == 0 else zi_prev[m][:, BLK - 1:BLK]
                        nc.vector.tensor_tensor_scan(
                            zr[:], rho_sb[m][:], gr[:], init_r, op0=op.mult, op1=op.add
                        )
                        nc.vector.tensor_tensor_scan(
                            zi[:], rho_sb[m][:], gi[:], init_i, op0=op.mult, op1=op.add
                        )
                        zr_prev[m], zi_prev[m] = zr, zi
                        # demodulate: x = e^{i th t} z
                        t5 = tpool.tile([128, BLK], dt, tag="t1")
                        t6 = tpool.tile([128, BLK], dt, tag="t2")
                        nc.vector.tensor_tensor(t5[:], ctt, zr[:], op=op.mult)
                        nc.vector.tensor_tensor(t6[:], stt, zi[:], op=op.mult)
                        xr = xrpool.tile([128, BLK], dt, tag=f"xr{m}")
                        nc.vector.tensor_tensor(xr[:], t5[:], t6[:], op=op.subtract)
                        t7 = tpool.tile([128, BLK], dt, tag="t7")
                        t8 = tpool.tile([128, BLK], dt, tag="t8")
                        nc.gpsimd.tensor_tensor(t7[:], stt, zr[:], op=op.mult)
                        nc.gpsimd.tensor_tensor(t8[:], ctt, zi[:], op=op.mult)
                        xi = xipool.tile([128, BLK], dt, tag=f"xi{m}")
                        nc.gpsimd.tensor_tensor(xi[:], t7[:], t8[:], op=op.add)
                        xr_blk[m], xi_blk[m] = xr, xi
                    # y = sum_m CtRe_m^T x_r[m] + (-CtIm_m)^T x_i[m] + D^T u
                    yps = ypool.tile([128, BLK], dt, tag="y")
                    for m in range(NT):
                        nc.tensor.matmul(
                            yps[:], ctr_sb[:, m * OUT:(m + 1) * OUT], xr_blk[m][:],
                            start=(m == 0), stop=False,
                        )
                        nc.tensor.matmul(
                            yps[:], cti_sb[:, m * OUT:(m + 1) * OUT], xi_blk[m][:],
                            start=False, stop=False,
                        )
                    nc.tensor.matmul(
                        yps[:], dwt_sb[:], ucols, start=False, stop=True
                    )
                    ysb = spool.tile([128, BLK], dt, tag="ysb")
                    nc.scalar.copy(ysb[:], yps[:])
                    nc.gpsimd.dma_start(yout[:, col0:col0 + BLK], ysb[:])

    _legalize_multi_waits(nc)
    return nc


def _legalize_multi_waits(nc):
    """This walrus build accepts a single sync wait per instruction; split
    any multi-wait instruction into same-engine single-wait NoOps + the
    original carrying the last wait (program order chains them)."""
    import bass_rust
    from concourse import mybir

    uid = [0]
    for fn in nc.m.functions:
        for bb in fn.blocks:
            insts = bb.instructions
            new = []
            changed = False
            for inst in insts:
                si = inst.sync_info
                if si is not None and len(si.on_wait) > 1:
                    waits = list(si.on_wait)
                    for w in waits[:-1]:
                        uid[0] += 1
                        new.append(mybir.InstNoOp(
                            name=f"mwsplit-{uid[0]}",
                            engine=inst.engine,
                            ins=[], outs=[],
                            sync_info=bass_rust.SyncInfo(on_wait=[w], on_update=[]),
                        ))
                    inst.sync_info = bass_rust.SyncInfo(
                        on_wait=[waits[-1]], on_update=list(si.on_update)
                    )
                    changed = True
                new.append(inst)
            if changed:
                bb.instructions = new


def _host_prep(A_re, A_im, B_re, B_im, C_re, C_im, D_w):
    """fp64 eigendecomposition + transposed/modulation-table layouts."""
    A = A_re.astype(np.float64) + 1j * A_im.astype(np.float64)
    w, V = np.linalg.eig(A)
    Vinv = np.linalg.inv(V)
    Bt = Vinv @ (B_re.astype(np.float64) + 1j * B_im.astype(np.float64))  # [N, IN]
    Ct = (C_re.astype(np.float64) + 1j * C_im.astype(np.float64)) @ V     # [OUT, N]

    rho = np.abs(w)
    theta = np.angle(w)
    tg = np.arange(1, T + 1, dtype=np.float64)
    ang = np.outer(theta, tg)  # [N, T]
    cost = np.cos(ang).astype(np.float32).reshape(NT, 128, T)
    sint = np.sin(ang).astype(np.float32).reshape(NT, 128, T)
    rho_b = np.broadcast_to(
        rho.astype(np.float32).reshape(NT, 128, 1), (NT, 128, BLK)
    ).copy()

    ctrT = np.ascontiguousarray(Ct.real.T, dtype=np.float32)   # [N, OUT]
    ctiT = np.ascontiguousarray(-Ct.imag.T, dtype=np.float32)  # [N, OUT]
    # shared blob columns (everything except the leading per-core ut block),
    # all [128, w]:
    def tbpiece(k):
        cs = cost[:, :, k * BLK:(k + 1) * BLK]  # [NT, 128, BLK]
        ss = sint[:, :, k * BLK:(k + 1) * BLK]
        return [np.ascontiguousarray(cs.transpose(1, 0, 2).reshape(128, NT * BLK)),
                np.ascontiguousarray(ss.transpose(1, 0, 2).reshape(128, NT * BLK))]
    parts = [
        np.ascontiguousarray(Bt.real.T, dtype=np.float32),  # [128(i), N]
        np.ascontiguousarray(Bt.imag.T, dtype=np.float32),
    ]
    parts += tbpiece(0)
    parts += [np.ascontiguousarray(D_w.T, dtype=np.float32)]
    parts += [np.ascontiguousarray(ctrT.reshape(NT, 128, OUT).transpose(1, 0, 2)
                                   .reshape(128, NT * OUT))]
    parts += [np.ascontiguousarray(ctiT.reshape(NT, 128, OUT).transpose(1, 0, 2)
                                   .reshape(128, NT * OUT))]
    parts += [np.ascontiguousarray(rho_b.transpose(1, 0, 2).reshape(128, NT * BLK))]
    for k in range(1, TBLK):
        parts += tbpiece(k)
    return np.concatenate(parts, axis=1)  # [128, BLOBW - COLS]


def _ensure_axon_hooks():
    """Provide antenv.axon_hooks if the image lacks it (needed only for
    trace=True NTFF profiling; run path works without)."""
    import types
    try:
        from antenv import axon_hooks  # noqa: F401
        return
    except ImportError:
        pass
    try:
        import antenv
        mod = types.ModuleType("antenv.axon_hooks")
        _hook = [None]
        mod.set_axon_ntff_profile_hook = lambda h: _hook.__setitem__(0, h)
        mod.get_axon_ntff_profile_hook = lambda: _hook[0]
        sys.modules["antenv.axon_hooks"] = mod
        antenv.axon_hooks = mod
        if "/root/.axon_site" not in sys.path:
            sys.path.insert(0, "/root/.axon_site")
        from trn_agent_boot.trn_boot import _ntff_profile_via_ctypes
        h = _ntff_profile_via_ctypes("/opt/axon/libaxon_pjrt.so")
        if h is not None:
            mod.set_axon_ntff_profile_hook(h)
    except Exception:
        pass


def kernel(u, A_re, A_im, B_re, B_im, C_re, C_im, D_w, output_bias):
    global LAST_RESULT, _NC_CACHE
    from concourse import bass_utils

    _ensure_axon_hooks()

    u = np.asarray(u, dtype=np.float32)
    shared = _host_prep(
        np.asarray(A_re), np.asarray(A_im), np.asarray(B_re), np.asarray(B_im),
        np.asarray(C_re), np.asarray(C_im), np.asarray(D_w)
    )

    if _NC_CACHE is None:
        _NC_CACHE = _build_nc()
    nc = _NC_CACHE

    in_maps = []
    for k in range(NCORES):
        u_pair = u[BLOCAL * k:BLOCAL * (k + 1)]  # [2, T, IN]
        ut = np.ascontiguousarray(
            u_pair.transpose(2, 0, 1).reshape(128, COLS), dtype=np.float32
        )
        in_maps.append({"blob": np.concatenate([ut, shared], axis=1)})

    res = bass_utils.run_bass_kernel_spmd(nc, in_maps, core_ids=list(range(NCORES)))
    LAST_RESULT = res

    y = np.empty((BATCH, T, OUT), dtype=np.float32)
    for k in range(NCORES):
        yd = res.results[k]["y"]  # [OUT, COLS]
        y[BLOCAL * k:BLOCAL * (k + 1)] = (
            yd.reshape(OUT, BLOCAL, T).transpose(1, 2, 0)
        )
    y += np.asarray(output_bias, dtype=np.float32)
    return y



# revision 6
# speedup vs baseline: 4.0555x; 4.0555x over previous
"""Trainium2 Bass kernel for nn_BaseLinearSSM (chunked formulation).

y[b,t] = Re(C @ x_{t+1}) + D @ u[b,t] + bias,  x_{t+1} = A x_t + B u_t  (complex A,B,C)

Strategy (chunk length L=8, NK=T/L=256 chunks):
  Host (fp64): eigendecompose A = V diag(w) V^-1, Bt = V^-1 B, Ct = C V.
  Precompute:
    Pt_j = diag(w^(L-1-j)) Bt          [N,IN]  (chunk input aggregation)
    Qt_j = Ct diag(w^(j+1))            [OUT,N] (chunk boundary -> outputs)
    K_d  = Re(C A^d B), K_0 += D       [OUT,IN] real (within-chunk causal conv)
  Device (per core, batch-sharded 2 of 16; everything bf16 except PSUM/scan state):
    phase 1: vt_k = sum_j Pt_j u_{kL+j}                    (matmuls, PSUM)
    phase 2: S_k = w^L S_{k-1} + vt_k  via modulate/scan/demodulate on the
             CHUNK axis only (T/L columns -> 1/8 the DVE work of a full scan);
             demod written with a one-chunk shift so S_shift[k] = beta_k =
             state at chunk start (col k=0 memset to 0 per batch element)
    phase 3: y_{kL+j} = Re(Qt_j beta_k) + sum_d K_d u_{kL+j-d}  (matmuls)
  Time is laid out (j, b, k) so every matmul has 512 contiguous columns.
  Host shards u, permutes layouts, gathers y, adds bias.
"""

import sys

import numpy as np

if "/opt/trn_rl_repo" not in sys.path:
    sys.path.insert(0, "/opt/trn_rl_repo")

BATCH, T, IN, OUT, N = 16, 2048, 128, 128, 512
NCORES = 8
BLOCAL = BATCH // NCORES   # 2
L = 8                      # chunk length
NK = T // L                # 256 chunks per batch element
NKB = BLOCAL * NK          # 512 chunk-columns per core (b-major)
NT = N // 128              # 4 partition tiles over the state dim
COLS = BLOCAL * T          # 4096

# blob16 (bf16) layout, DMA piece order = consumption order:
#   p0: u_jk [L*NKB = 4096]
#   p1..p4 (per m): PtT[m] ([ri][j] -> 16*128 = 2048) | ck2[m] (512) | sk2[m] (512)
#   p5: KT (8*128 = 1024) | QtT ([j][ri][m] -> 64*128 = 8192)
UW = L * NKB               # 4096
PMW = 2 * L * 128 + 2 * NKB  # 3072 per m
P5W = L * 128 + L * 2 * NT * 128  # 9216
W16 = UW + NT * PMW + P5W  # 25600
W32 = NT * NKB             # rho2 tables (fp32), 2048

LAST_RESULT = None
_NC_CACHE = None


def _build_nc():
    from concourse import bass, mybir
    from concourse import tile

    f32 = mybir.dt.float32
    bf16 = mybir.dt.bfloat16
    op = mybir.AluOpType

    nc = bass.Bass("TRN2", target_bir_lowering=False, debug=False)

    blob16 = nc.dram_tensor("blob16", [128, W16], bf16, kind="ExternalInput")
    blob32 = nc.dram_tensor("blob32", [128, W32], f32, kind="ExternalInput")
    yout = nc.dram_tensor("y", [OUT, COLS], f32, kind="ExternalOutput")

    with tile.TileContext(nc) as tc:
        with (
            tc.tile_pool(name="const", bufs=1) as cpool,
            tc.tile_pool(name="vsb", bufs=2) as vpool,
            tc.tile_pool(name="tmp", bufs=2) as tpool,
            tc.tile_pool(name="gz", bufs=2) as gpool,
            tc.tile_pool(name="S", bufs=1) as spool,
            tc.tile_pool(name="ysb", bufs=2) as ypool_sb,
            tc.tile_pool(name="vps", bufs=2, space="PSUM") as vtpool,
            tc.tile_pool(name="yps", bufs=2, space="PSUM") as ypool,
        ):
            b16 = cpool.tile([128, W16], bf16)
            b32 = cpool.tile([128, W32], f32)
            # DMA pieces in dependency order
            bounds = [0, UW]
            for m in range(NT):
                bounds.append(bounds[-1] + PMW)
            bounds.append(bounds[-1] + P5W)
            for a, b in zip(bounds[:-1], bounds[1:]):
                nc.sync.dma_start(b16[:, a:b], blob16[:, a:b])
            nc.sync.dma_start(b32[:, :], blob32[:, :])

            o = [0]

            def take(w):
                s = b16[:, o[0]:o[0] + w]
                o[0] += w
                return s

            u_jk = take(UW)
            ptT = [[[None] * L for _ in range(2)] for _ in range(NT)]
            ck2 = [None] * NT
            sk2 = [None] * NT
            for m in range(NT):
                for ri in range(2):
                    for j in range(L):
                        ptT[m][ri][j] = take(128)
                ck2[m] = take(NKB)
                sk2[m] = take(NKB)
            ktT = [take(128) for _ in range(L)]
            qtT = [[[None] * NT for _ in range(2)] for _ in range(L)]
            for j in range(L):
                for ri in range(2):
                    for m in range(NT):
                        qtT[j][ri][m] = take(128)
            assert o[0] == W16
            rho2 = [b32[:, m * NKB:(m + 1) * NKB] for m in range(NT)]

            Sr_t = [None] * NT
            Si_t = [None] * NT
            for m in range(NT):
                # phase 1: vt = sum_j Pt_j u_j  (complex, PSUM)
                v_sb = [None, None]
                for ri in range(2):
                    vt = vtpool.tile([128, NKB], f32, tag=f"vt{ri}")
                    for j in range(L):
                        nc.tensor.matmul(
                            vt[:], ptT[m][ri][j], u_jk[:, j * NKB:(j + 1) * NKB],
                            start=(j == 0), stop=(j == L - 1),
                        )
                    v_sb[ri] = vpool.tile([128, NKB], bf16, tag=f"v{ri}", name=f"v{ri}")
                    nc.scalar.copy(v_sb[ri][:], vt[:])
                vr, vi = v_sb
                # phase 2: modulate  g = e^{-i phi (k+1)} vt
                t1 = tpool.tile([128, NKB], bf16, tag="t1")
                t2 = tpool.tile([128, NKB], bf16, tag="t2")
                nc.vector.tensor_tensor(t1[:], ck2[m], vr[:], op=op.mult)
                nc.vector.tensor_tensor(t2[:], sk2[m], vi[:], op=op.mult)
                gr = gpool.tile([128, NKB], bf16, tag="gr")
                nc.vector.tensor_tensor(gr[:], t1[:], t2[:], op=op.add)
                t3 = tpool.tile([128, NKB], bf16, tag="t3")
                t4 = tpool.tile([128, NKB], bf16, tag="t4")
                nc.gpsimd.tensor_tensor(t3[:], ck2[m], vi[:], op=op.mult)
                nc.gpsimd.tensor_tensor(t4[:], sk2[m], vr[:], op=op.mult)
                gi = gpool.tile([128, NKB], bf16, tag="gi")
                nc.gpsimd.tensor_tensor(gi[:], t3[:], t4[:], op=op.subtract)
                # scan along k; rho2 has col NK zeroed to reset state at the
                # second batch element (fp32 state, bf16 IO)
                zr = gpool.tile([128, NKB], bf16, tag="zr")
                zi = gpool.tile([128, NKB], bf16, tag="zi")
                nc.vector.tensor_tensor_scan(
                    zr[:], rho2[m], gr[:], 0.0, op0=op.mult, op1=op.add
                )
                nc.vector.tensor_tensor_scan(
                    zi[:], rho2[m], gi[:], 0.0, op0=op.mult, op1=op.add
                )
                # demodulate S = e^{i phi (k+1)} z, written shifted one chunk:
                # S_shift[:, b*NK + k] = S_{k-1} (= beta_k), col k=0 zeroed
                t5 = tpool.tile([128, NKB], bf16, tag="t5")
                t6 = tpool.tile([128, NKB], bf16, tag="t6")
                nc.vector.tensor_tensor(t5[:], ck2[m], zr[:], op=op.mult)
                nc.vector.tensor_tensor(t6[:], sk2[m], zi[:], op=op.mult)
                t7 = tpool.tile([128, NKB], bf16, tag="t7")
                t8 = tpool.tile([128, NKB], bf16, tag="t8")
                nc.gpsimd.tensor_tensor(t7[:], sk2[m], zr[:], op=op.mult)
                nc.gpsimd.tensor_tensor(t8[:], ck2[m], zi[:], op=op.mult)
                Sr = spool.tile([128, NKB], bf16, tag=f"Sr{m}")
                Si = spool.tile([128, NKB], bf16, tag=f"Si{m}")
                nc.vector.memset(Sr[:, 0:1], 0.0)
                nc.vector.memset(Sr[:, NK:NK + 1], 0.0)
                nc.gpsimd.memset(Si[:, 0:1], 0.0)
                nc.gpsimd.memset(Si[:, NK:NK + 1], 0.0)
                for b in range(BLOCAL):
                    a0 = b * NK
                    nc.vector.tensor_tensor(
                        Sr[:, a0 + 1:a0 + NK], t5[:, a0:a0 + NK - 1],
                        t6[:, a0:a0 + NK - 1], op=op.subtract,
                    )
                    nc.gpsimd.tensor_tensor(
                        Si[:, a0 + 1:a0 + NK], t7[:, a0:a0 + NK - 1],
                        t8[:, a0:a0 + NK - 1], op=op.add,
                    )
                Sr_t[m], Si_t[m] = Sr, Si

            # phase 3: per output slot j, y = conv + boundary terms
            for j in range(L):
                yps = ypool.tile([128, NKB], f32, tag="y")
                for d in range(j + 1):
                    nc.tensor.matmul(
                        yps[:], ktT[d], u_jk[:, (j - d) * NKB:(j - d + 1) * NKB],
                        start=(d == 0), stop=False,
                    )
                for m in range(NT):
                    nc.tensor.matmul(
                        yps[:], qtT[j][0][m], Sr_t[m][:], start=False, stop=False,
                    )
                    nc.tensor.matmul(
                        yps[:], qtT[j][1][m], Si_t[m][:], start=False,
                        stop=(m == NT - 1),
                    )
                ysb = ypool_sb.tile([128, NKB], f32, tag="ysb")
                nc.scalar.copy(ysb[:], yps[:])
                nc.gpsimd.dma_start(yout[:, j * NKB:(j + 1) * NKB], ysb[:])

    _legalize_multi_waits(nc)
    return nc


def _legalize_multi_waits(nc):
    """This walrus build accepts a single sync wait per instruction; split
    any multi-wait instruction into same-engine single-wait NoOps + the
    original carrying the last wait (program order chains them)."""
    import bass_rust
    from concourse import mybir

    uid = [0]
    for fn in nc.m.functions:
        for bb in fn.blocks:
            insts = bb.instructions
            new = []
            changed = False
            for inst in insts:
                si = inst.sync_info
                if si is not None and len(si.on_wait) > 1:
                    waits = list(si.on_wait)
                    for w in waits[:-1]:
                        uid[0] += 1
                        new.append(mybir.InstNoOp(
                            name=f"mwsplit-{uid[0]}",
                            engine=inst.engine,
                            ins=[], outs=[],
                            sync_info=bass_rust.SyncInfo(on_wait=[w], on_update=[]),
                        ))
                    inst.sync_info = bass_rust.SyncInfo(
                        on_wait=[waits[-1]], on_update=list(si.on_update)
                    )
                    changed = True
                new.append(inst)
            if changed:
                bb.instructions = new


def _host_prep(A_re, A_im, B_re, B_im, C_re, C_im, D_w):
    """fp64 eigendecomposition + chunked-formulation weight/table layouts.
    Returns (shared16 [128, W16-UW] bf16, blob32 [128, W32] f32)."""
    import ml_dtypes

    bf = ml_dtypes.bfloat16
    A = A_re.astype(np.float64) + 1j * A_im.astype(np.float64)
    w, V = np.linalg.eig(A)
    Vinv = np.linalg.inv(V)
    Bt = Vinv @ (B_re.astype(np.float64) + 1j * B_im.astype(np.float64))
    Ct = (C_re.astype(np.float64) + 1j * C_im.astype(np.float64)) @ V

    Pt = np.stack([(w ** (L - 1 - j))[:, None] * Bt for j in range(L)])  # [L,N,IN]
    Qt = np.stack([Ct * (w ** (j + 1))[None, :] for j in range(L)])      # [L,OUT,N]
    K = np.empty((L, OUT, IN))
    Ad = np.eye(N, dtype=complex)
    Bc = B_re.astype(np.float64) + 1j * B_im.astype(np.float64)
    Cc = C_re.astype(np.float64) + 1j * C_im.astype(np.float64)
    for d in range(L):
        K[d] = (Cc @ Ad @ Bc).real
        Ad = A @ Ad
    K[0] += D_w.astype(np.float64)

    wL = w ** L
    rhoL = np.abs(wL)
    phi = np.angle(wL)
    kk = np.arange(NK)
    cosk = np.cos(np.outer(phi, kk + 1))  # [N, NK]
    sink = np.sin(np.outer(phi, kk + 1))

    parts16 = []
    for m in range(NT):
        sl = slice(m * 128, (m + 1) * 128)
        for Pp in (Pt.real, Pt.imag):
            for j in range(L):
                parts16.append(np.ascontiguousarray(Pp[j].T[:, sl]))  # [IN, 128]
        parts16.append(np.tile(cosk[sl], (1, BLOCAL)))  # [128, NKB]
        parts16.append(np.tile(sink[sl], (1, BLOCAL)))
    for d in range(L):
        parts16.append(np.ascontiguousarray(K[d].T))  # [IN, OUT]
    for j in range(L):
        for Qp in (Qt[j].real, -Qt[j].imag):
            QT = np.ascontiguousarray(Qp.T)  # [N, OUT]
            for m in range(NT):
                parts16.append(QT[m * 128:(m + 1) * 128])
    shared16 = np.concatenate(parts16, axis=1).astype(bf)
    assert shared16.shape == (128, W16 - UW)

    rho2 = np.empty((128, W32), dtype=np.float32)
    for m in range(NT):
        rb = np.broadcast_to(
            rhoL[m * 128:(m + 1) * 128].astype(np.float32)[:, None], (128, NKB)
        ).copy()
        rb[:, NK] = 0.0  # reset scan state at second batch element
        rho2[:, m * NKB:(m + 1) * NKB] = rb
    return shared16, rho2


def _ensure_axon_hooks():
    """Provide antenv.axon_hooks if the image lacks it (needed only for
    trace=True NTFF profiling; run path works without)."""
    import types
    try:
        from antenv import axon_hooks  # noqa: F401
        return
    except ImportError:
        pass
    try:
        import antenv
        mod = types.ModuleType("antenv.axon_hooks")
        _hook = [None]
        mod.set_axon_ntff_profile_hook = lambda h: _hook.__setitem__(0, h)
        mod.get_axon_ntff_profile_hook = lambda: _hook[0]
        sys.modules["antenv.axon_hooks"] = mod
        antenv.axon_hooks = mod
        if "/root/.axon_site" not in sys.path:
            sys.path.insert(0, "/root/.axon_site")
        from trn_agent_boot.trn_boot import _ntff_profile_via_ctypes
        h = _ntff_profile_via_ctypes("/opt/axon/libaxon_pjrt.so")
        if h is not None:
            mod.set_axon_ntff_profile_hook(h)
    except Exception:
        pass


def kernel(u, A_re, A_im, B_re, B_im, C_re, C_im, D_w, output_bias):
    global LAST_RESULT, _NC_CACHE
    import ml_dtypes
    from concourse import bass_utils

    _ensure_axon_hooks()

    bf = ml_dtypes.bfloat16
    u = np.asarray(u, dtype=np.float32)
    shared16, rho2 = _host_prep(
        np.asarray(A_re), np.asarray(A_im), np.asarray(B_re), np.asarray(B_im),
        np.asarray(C_re), np.asarray(C_im), np.asarray(D_w)
    )

    if _NC_CACHE is None:
        _NC_CACHE = _build_nc()
    nc = _NC_CACHE

    in_maps = []
    for c in range(NCORES):
        up = u[BLOCAL * c:BLOCAL * (c + 1)]           # [2, T, IN]
        uc = up.reshape(BLOCAL, NK, L, IN)            # t = k*L + j
        u_jk = np.ascontiguousarray(
            uc.transpose(3, 2, 0, 1).reshape(IN, L * NKB)
        ).astype(bf)                                  # col = j*NKB + b*NK + k
        in_maps.append({
            "blob16": np.concatenate([u_jk, shared16], axis=1),
            "blob32": rho2,
        })

    res = bass_utils.run_bass_kernel_spmd(nc, in_maps, core_ids=list(range(NCORES)))
    LAST_RESULT = res

    y = np.empty((BATCH, T, OUT), dtype=np.float32)
    for c in range(NCORES):
        yd = res.results[c]["y"]                      # [OUT, L*NKB]
        y[BLOCAL * c:BLOCAL * (c + 1)] = (
            yd.reshape(OUT, L, BLOCAL, NK).transpose(2, 3, 1, 0)
            .reshape(BLOCAL, T, OUT)
        )
    y += np.asarray(output_bias, dtype=np.float32)
    return y


# revision 11
# speedup vs baseline: 4.2508x; 1.0482x over previous
"""Trainium2 Bass kernel for nn_BaseLinearSSM (chunked formulation).

y[b,t] = Re(C @ x_{t+1}) + D @ u[b,t] + bias,  x_{t+1} = A x_t + B u_t  (complex A,B,C)

Strategy (chunk length L=8, NK=T/L=256 chunks):
  Host (fp64): eigendecompose A = V diag(w) V^-1, Bt = V^-1 B, Ct = C V.
  Precompute:
    Pt_j = diag(w^(L-1-j)) Bt          [N,IN]  (chunk input aggregation)
    Qt_j = Ct diag(w^(j+1))            [OUT,N] (chunk boundary -> outputs)
    K_d  = Re(C A^d B), K_0 += D       [OUT,IN] real (within-chunk causal conv)
  Device (per core, batch-sharded 2 of 16; everything bf16 except PSUM/scan state):
    phase 1: vt_k = sum_j Pt_j u_{kL+j}                    (matmuls, PSUM)
    phase 2: S_k = w^L S_{k-1} + vt_k  via modulate/scan/demodulate on the
             CHUNK axis only (T/L columns -> 1/8 the DVE work of a full scan);
             demod written with a one-chunk shift so S_shift[k] = beta_k =
             state at chunk start (col k=0 memset to 0 per batch element)
    phase 3: y_{kL+j} = Re(Qt_j beta_k) + sum_d K_d u_{kL+j-d}  (matmuls)
  Time is laid out (j, b, k) so every matmul has 512 contiguous columns.
  Host shards u, permutes layouts, gathers y, adds bias.
"""

import sys

import numpy as np

if "/opt/trn_rl_repo" not in sys.path:
    sys.path.insert(0, "/opt/trn_rl_repo")

BATCH, T, IN, OUT, N = 16, 2048, 128, 128, 512
NCORES = 8
BLOCAL = BATCH // NCORES   # 2
L = 8                      # chunk length
NK = T // L                # 256 chunks per batch element
NKB = BLOCAL * NK          # 512 chunk-columns per core (b-major)
NT = N // 128              # 4 partition tiles over the state dim
COLS = BLOCAL * T          # 4096

# blob16 (bf16) layout, DMA piece order = consumption order:
#   u_jk [4096] | KT [1024] | Pt0 | Pt1 | tab0 | Pt2 | tab1 | Pt3 | tab2 |
#   tab3 | QtT [8192]   (PtM = [ri][j] 16*128 = 2048, tabM = ck2|sk2 = 1024)
UW = L * NKB               # 4096
KW = L * 128               # 1024
PW = 2 * L * 128           # 2048 per m
TW = 2 * NKB               # 1024 per m
QW = L * 2 * NT * 128      # 8192
W16 = UW + KW + NT * (PW + TW) + QW  # 25600
W32 = NT * NKB             # rho2 tables (fp32), 2048

LAST_RESULT = None
_NC_CACHE = None


def _build_nc():
    from concourse import bass, mybir
    from concourse import tile

    f32 = mybir.dt.float32
    bf16 = mybir.dt.bfloat16
    op = mybir.AluOpType

    nc = bass.Bass("TRN2", target_bir_lowering=False, debug=False)

    blob16 = nc.dram_tensor("blob16", [128, W16], bf16, kind="ExternalInput")
    blob32 = nc.dram_tensor("blob32", [128, W32], f32, kind="ExternalInput")
    yout = nc.dram_tensor("y", [OUT, COLS], f32, kind="ExternalOutput")

    with tile.TileContext(nc) as tc:
        with (
            tc.tile_pool(name="const", bufs=1) as cpool,
            tc.tile_pool(name="vsb", bufs=2) as vpool,
            tc.tile_pool(name="tmp", bufs=2) as tpool,
            tc.tile_pool(name="gz", bufs=2) as gpool,
            tc.tile_pool(name="S", bufs=1) as spool,
            tc.tile_pool(name="ysb", bufs=4) as ypool_sb,
            tc.tile_pool(name="ps", bufs=1, space="PSUM") as pspool,
        ):
            b16 = cpool.tile([128, W16], bf16)
            b32 = cpool.tile([128, W32], f32)
            # DMA pieces in dependency order:
            # u | K | Pt0 | Pt1 | tab0 | Pt2 | tab1 | Pt3 | tab2 | tab3 | Qt
            o = [0]

            def take(w):
                s = b16[:, o[0]:o[0] + w]
                o[0] += w
                return s

            u_jk = take(UW)
            ktT = [take(128) for _ in range(L)]
            ptT = [[[None] * L for _ in range(2)] for _ in range(NT)]
            ck2 = [None] * NT
            sk2 = [None] * NT

            def take_pt(m):
                for ri in range(2):
                    for j in range(L):
                        ptT[m][ri][j] = take(128)

            def take_tab(m):
                ck2[m] = take(NKB)
                sk2[m] = take(NKB)

            take_pt(0)
            take_pt(1)
            take_tab(0)
            take_pt(2)
            take_tab(1)
            take_pt(3)
            take_tab(2)
            take_tab(3)
            qtT = [[[None] * NT for _ in range(2)] for _ in range(L)]
            for j in range(L):
                for ri in range(2):
                    for m in range(NT):
                        qtT[j][ri][m] = take(128)
            assert o[0] == W16
            rho2 = [b32[:, m * NKB:(m + 1) * NKB] for m in range(NT)]

            bounds = [0, UW + KW, UW + KW + PW, UW + KW + 2 * PW,
                      UW + KW + 2 * PW + TW, UW + KW + 3 * PW + TW,
                      UW + KW + 3 * PW + 2 * TW, UW + KW + 4 * PW + 2 * TW,
                      UW + KW + 4 * PW + 3 * TW, UW + KW + 4 * PW + 4 * TW, W16]
            for a, b in zip(bounds[:-1], bounds[1:]):
                nc.sync.dma_start(b16[:, a:b], blob16[:, a:b])
            nc.sync.dma_start(b32[:, :], blob32[:, :])

            Sr_t = [None] * NT
            Si_t = [None] * NT
            for m in range(NT):
                # phase 1: vt = sum_j Pt_j u_j  (complex, PSUM)
                v_sb = [None, None]
                for ri in range(2):
                    vt = pspool.tile([128, NKB], f32, tag=f"vt{ri}", bufs=2,
                                     name=f"vt{ri}")
                    for j in range(L):
                        nc.tensor.matmul(
                            vt[:], ptT[m][ri][j], u_jk[:, j * NKB:(j + 1) * NKB],
                            start=(j == 0), stop=(j == L - 1),
                        )
                    v_sb[ri] = vpool.tile([128, NKB], bf16, tag=f"v{ri}", name=f"v{ri}")
                    nc.scalar.copy(v_sb[ri][:], vt[:])
                vr, vi = v_sb
                # phase 2: modulate  g = e^{-i phi (k+1)} vt
                t1 = tpool.tile([128, NKB], bf16, tag="t1")
                t2 = tpool.tile([128, NKB], bf16, tag="t2")
                nc.vector.tensor_tensor(t1[:], ck2[m], vr[:], op=op.mult)
                nc.vector.tensor_tensor(t2[:], sk2[m], vi[:], op=op.mult)
                gr = gpool.tile([128, NKB], bf16, tag="gr")
                nc.vector.tensor_tensor(gr[:], t1[:], t2[:], op=op.add)
                t3 = tpool.tile([128, NKB], bf16, tag="t3")
                t4 = tpool.tile([128, NKB], bf16, tag="t4")
                nc.gpsimd.tensor_tensor(t3[:], ck2[m], vi[:], op=op.mult)
                nc.gpsimd.tensor_tensor(t4[:], sk2[m], vr[:], op=op.mult)
                gi = gpool.tile([128, NKB], bf16, tag="gi")
                nc.gpsimd.tensor_tensor(gi[:], t3[:], t4[:], op=op.subtract)
                # scan along k; rho2 has col NK zeroed to reset state at the
                # second batch element (fp32 state, bf16 IO)
                zr = gpool.tile([128, NKB], bf16, tag="zr")
                zi = gpool.tile([128, NKB], bf16, tag="zi")
                nc.vector.tensor_tensor_scan(
                    zr[:], rho2[m], gr[:], 0.0, op0=op.mult, op1=op.add
                )
                nc.vector.tensor_tensor_scan(
                    zi[:], rho2[m], gi[:], 0.0, op0=op.mult, op1=op.add
                )
                # demodulate S = e^{i phi (k+1)} z, written shifted one chunk:
                # S_shift[:, b*NK + k] = S_{k-1} (= beta_k), col k=0 zeroed
                t5 = tpool.tile([128, NKB], bf16, tag="t5")
                t6 = tpool.tile([128, NKB], bf16, tag="t6")
                nc.vector.tensor_tensor(t5[:], ck2[m], zr[:], op=op.mult)
                nc.vector.tensor_tensor(t6[:], sk2[m], zi[:], op=op.mult)
                t7 = tpool.tile([128, NKB], bf16, tag="t7")
                t8 = tpool.tile([128, NKB], bf16, tag="t8")
                nc.gpsimd.tensor_tensor(t7[:], sk2[m], zr[:], op=op.mult)
                nc.gpsimd.tensor_tensor(t8[:], ck2[m], zi[:], op=op.mult)
                Sr = spool.tile([128, NKB], bf16, tag=f"Sr{m}")
                Si = spool.tile([128, NKB], bf16, tag=f"Si{m}")
                nc.vector.memset(Sr[:, 0:1], 0.0)
                nc.vector.memset(Sr[:, NK:NK + 1], 0.0)
                nc.gpsimd.memset(Si[:, 0:1], 0.0)
                nc.gpsimd.memset(Si[:, NK:NK + 1], 0.0)
                for b in range(BLOCAL):
                    a0 = b * NK
                    nc.vector.tensor_tensor(
                        Sr[:, a0 + 1:a0 + NK], t5[:, a0:a0 + NK - 1],
                        t6[:, a0:a0 + NK - 1], op=op.subtract,
                    )
                    nc.gpsimd.tensor_tensor(
                        Si[:, a0 + 1:a0 + NK], t7[:, a0:a0 + NK - 1],
                        t8[:, a0:a0 + NK - 1], op=op.add,
                    )
                Sr_t[m], Si_t[m] = Sr, Si

            # phase 3: two waves of 4 output slots j. Within a wave: all conv
            # matmuls first (depend only on u/K -> keep PE busy), then the
            # boundary matmuls ordered m-outer so the ones needing the last
            # S tiles issue last (phase-2 tail overlap).
            for wave in range(2):
                js = range(wave * 4, wave * 4 + 4)
                yps = {}
                for j in js:
                    yps[j] = pspool.tile([128, NKB], f32, tag="y", bufs=4,
                                         name=f"y{j}")
                    for d in range(j + 1):
                        nc.tensor.matmul(
                            yps[j][:], ktT[d],
                            u_jk[:, (j - d) * NKB:(j - d + 1) * NKB],
                            start=(d == 0), stop=False, skip_group_check=True,
                        )
                for m in range(NT):
                    for j in js:
                        nc.tensor.matmul(
                            yps[j][:], qtT[j][0][m], Sr_t[m][:], start=False,
                            stop=False, skip_group_check=True,
                        )
                        nc.tensor.matmul(
                            yps[j][:], qtT[j][1][m], Si_t[m][:], start=False,
                            stop=(m == NT - 1), skip_group_check=True,
                        )
                for j in js:
                    ysb = ypool_sb.tile([128, NKB], f32, tag="ysb")
                    nc.scalar.copy(ysb[:], yps[j][:])
                    nc.gpsimd.dma_start(yout[:, j * NKB:(j + 1) * NKB], ysb[:])

    _legalize_multi_waits(nc)
    return nc


def _legalize_multi_waits(nc):
    """This walrus build accepts a single sync wait per instruction; split
    any multi-wait instruction into same-engine single-wait NoOps + the
    original carrying the last wait (program order chains them)."""
    import bass_rust
    from concourse import mybir

    uid = [0]
    for fn in nc.m.functions:
        for bb in fn.blocks:
            insts = bb.instructions
            new = []
            changed = False
            for inst in insts:
                si = inst.sync_info
                if si is not None and len(si.on_wait) > 1:
                    waits = list(si.on_wait)
                    for w in waits[:-1]:
                        uid[0] += 1
                        new.append(mybir.InstNoOp(
                            name=f"mwsplit-{uid[0]}",
                            engine=inst.engine,
                            ins=[], outs=[],
                            sync_info=bass_rust.SyncInfo(on_wait=[w], on_update=[]),
                        ))
                    inst.sync_info = bass_rust.SyncInfo(
                        on_wait=[waits[-1]], on_update=list(si.on_update)
                    )
                    changed = True
                new.append(inst)
            if changed:
                bb.instructions = new


def _host_prep(A_re, A_im, B_re, B_im, C_re, C_im, D_w):
    """fp64 eigendecomposition + chunked-formulation weight/table layouts.
    Returns (shared16 [128, W16-UW] bf16, blob32 [128, W32] f32)."""
    import ml_dtypes

    bf = ml_dtypes.bfloat16
    A = A_re.astype(np.float64) + 1j * A_im.astype(np.float64)
    w, V = np.linalg.eig(A)
    Vinv = np.linalg.inv(V)
    Bt = Vinv @ (B_re.astype(np.float64) + 1j * B_im.astype(np.float64))
    Ct = (C_re.astype(np.float64) + 1j * C_im.astype(np.float64)) @ V

    Pt = np.stack([(w ** (L - 1 - j))[:, None] * Bt for j in range(L)])  # [L,N,IN]
    Qt = np.stack([Ct * (w ** (j + 1))[None, :] for j in range(L)])      # [L,OUT,N]
    K = np.empty((L, OUT, IN))
    Ad = np.eye(N, dtype=complex)
    Bc = B_re.astype(np.float64) + 1j * B_im.astype(np.float64)
    Cc = C_re.astype(np.float64) + 1j * C_im.astype(np.float64)
    for d in range(L):
        K[d] = (Cc @ Ad @ Bc).real
        Ad = A @ Ad
    K[0] += D_w.astype(np.float64)

    wL = w ** L
    rhoL = np.abs(wL)
    phi = np.angle(wL)
    kk = np.arange(NK)
    cosk = np.cos(np.outer(phi, kk + 1))  # [N, NK]
    sink = np.sin(np.outer(phi, kk + 1))

    parts16 = []
    for d in range(L):
        parts16.append(np.ascontiguousarray(K[d].T))  # [IN, OUT]

    def pt_piece(m):
        sl = slice(m * 128, (m + 1) * 128)
        for Pp in (Pt.real, Pt.imag):
            for j in range(L):
                parts16.append(np.ascontiguousarray(Pp[j].T[:, sl]))  # [IN, 128]

    def tab_piece(m):
        sl = slice(m * 128, (m + 1) * 128)
        parts16.append(np.tile(cosk[sl], (1, BLOCAL)))  # [128, NKB]
        parts16.append(np.tile(sink[sl], (1, BLOCAL)))

    pt_piece(0)
    pt_piece(1)
    tab_piece(0)
    pt_piece(2)
    tab_piece(1)
    pt_piece(3)
    tab_piece(2)
    tab_piece(3)
    for j in range(L):
        for Qp in (Qt[j].real, -Qt[j].imag):
            QT = np.ascontiguousarray(Qp.T)  # [N, OUT]
            for m in range(NT):
                parts16.append(QT[m * 128:(m + 1) * 128])
    shared16 = np.concatenate(parts16, axis=1).astype(bf)
    assert shared16.shape == (128, W16 - UW)

    rho2 = np.empty((128, W32), dtype=np.float32)
    for m in range(NT):
        rb = np.broadcast_to(
            rhoL[m * 128:(m + 1) * 128].astype(np.float32)[:, None], (128, NKB)
        ).copy()
        rb[:, NK] = 0.0  # reset scan state at second batch element
        rho2[:, m * NKB:(m + 1) * NKB] = rb
    return shared16, rho2


def _ensure_axon_hooks():
    """Provide antenv.axon_hooks if the image lacks it (needed only for
    trace=True NTFF profiling; run path works without)."""
    import types
    try:
        from antenv import axon_hooks  # noqa: F401
        return
    except ImportError:
        pass
    try:
        import antenv
        mod = types.ModuleType("antenv.axon_hooks")
        _hook = [None]
        mod.set_axon_ntff_profile_hook = lambda h: _hook.__setitem__(0, h)
        mod.get_axon_ntff_profile_hook = lambda: _hook[0]
        sys.modules["antenv.axon_hooks"] = mod
        antenv.axon_hooks = mod
        if "/root/.axon_site" not in sys.path:
            sys.path.insert(0, "/root/.axon_site")
        from trn_agent_boot.trn_boot import _ntff_profile_via_ctypes
        h = _ntff_profile_via_ctypes("/opt/axon/libaxon_pjrt.so")
        if h is not None:
            mod.set_axon_ntff_profile_hook(h)
    except Exception:
        pass


def kernel(u, A_re, A_im, B_re, B_im, C_re, C_im, D_w, output_bias):
    global LAST_RESULT, _NC_CACHE
    import ml_dtypes
    from concourse import bass_utils

    _ensure_axon_hooks()

    bf = ml_dtypes.bfloat16
    u = np.asarray(u, dtype=np.float32)
    shared16, rho2 = _host_prep(
        np.asarray(A_re), np.asarray(A_im), np.asarray(B_re), np.asarray(B_im),
        np.asarray(C_re), np.asarray(C_im), np.asarray(D_w)
    )

    if _NC_CACHE is None:
        _NC_CACHE = _build_nc()
    nc = _NC_CACHE

    in_maps = []
    for c in range(NCORES):
        up = u[BLOCAL * c:BLOCAL * (c + 1)]           # [2, T, IN]
        uc = up.reshape(BLOCAL, NK, L, IN)            # t = k*L + j
        u_jk = np.ascontiguousarray(
            uc.transpose(3, 2, 0, 1).reshape(IN, L * NKB)
        ).astype(bf)                                  # col = j*NKB + b*NK + k
        in_maps.append({
            "blob16": np.concatenate([u_jk, shared16], axis=1),
            "blob32": rho2,
        })

    res = bass_utils.run_bass_kernel_spmd(nc, in_maps, core_ids=list(range(NCORES)))
    LAST_RESULT = res

    y = np.empty((BATCH, T, OUT), dtype=np.float32)
    for c in range(NCORES):
        yd = res.results[c]["y"]                      # [OUT, L*NKB]
        y[BLOCAL * c:BLOCAL * (c + 1)] = (
            yd.reshape(OUT, L, BLOCAL, NK).transpose(2, 3, 1, 0)
            .reshape(BLOCAL, T, OUT)
        )
    y += np.asarray(output_bias, dtype=np.float32)
    return y


# revision 12
# speedup vs baseline: 4.7335x; 1.1136x over previous
"""Trainium2 Bass kernel for nn_BaseLinearSSM (chunked formulation).

y[b,t] = Re(C @ x_{t+1}) + D @ u[b,t] + bias,  x_{t+1} = A x_t + B u_t  (complex A,B,C)

Strategy (chunk length L=8, NK=T/L=256 chunks):
  Host (fp64): eigendecompose A = V diag(w) V^-1, Bt = V^-1 B, Ct = C V.
  Precompute:
    Pt_j = diag(w^(L-1-j)) Bt          [N,IN]  (chunk input aggregation)
    Qt_j = Ct diag(w^(j+1))            [OUT,N] (chunk boundary -> outputs)
    K_d  = Re(C A^d B), K_0 += D       [OUT,IN] real (within-chunk causal conv)
  Device (per core, batch-sharded 2 of 16; fp16 data, fp32 PSUM/scan state):
    phase 1: vt_k = sum_j Pt_j u_{kL+j}                    (matmuls, PSUM)
    phase 2: S_k = w^L S_{k-1} + vt_k  via modulate/scan/demodulate on the
             CHUNK axis only (T/L columns -> 1/8 the DVE work of a full scan);
             demod written with a one-chunk shift so S_shift[k] = beta_k =
             state at chunk start (col k=0 memset to 0 per batch element)
    phase 3: y_{kL+j} = Re(Qt_j beta_k) + sum_d K_d u_{kL+j-d}  (matmuls)
  Time is laid out (j, b, k) so every matmul has 512 contiguous columns.
  Phase 3 runs in two waves (j0..5, j6..7) with the boundary matmuls ordered
  m-outer, so the tensor engine only needs the last S tiles at the very end
  of wave A (phase-2 tail hidden behind conv + earlier-m matmuls).
  Input DMA is split over the two HWDGE rings (sync + scalar queues).
  Host shards u, permutes layouts, gathers y, adds bias.
"""

import sys

import numpy as np

if "/opt/trn_rl_repo" not in sys.path:
    sys.path.insert(0, "/opt/trn_rl_repo")

BATCH, T, IN, OUT, N = 16, 2048, 128, 128, 512
NCORES = 8
BLOCAL = BATCH // NCORES   # 2
L = 8                      # chunk length
NK = T // L                # 256 chunks per batch element
NKB = BLOCAL * NK          # 512 chunk-columns per core (b-major)
NT = N // 128              # 4 partition tiles over the state dim
COLS = BLOCAL * T          # 4096

# blob (fp16) layout:
#   u_jk [4096] | KT [1024] | per m: (PtT[m] [16*128] | ck2 | sk2 | rho2[m]) |
#   QtT [8192]
UW = L * NKB               # 4096
KW = L * 128               # 1024
PW = 2 * L * 128           # 2048 per m
TW = 2 * NKB               # 1024 per m (cos+sin)
RW = NKB                   # 512 per m (rho, col NK zeroed)
QW = L * 2 * NT * 128      # 8192
MW = PW + TW + RW          # 3584 per m
W16 = UW + KW + NT * MW + QW  # 27648

LAST_RESULT = None
_NC_CACHE = None


def _build_nc():
    from concourse import bass, mybir
    from concourse import tile

    f32 = mybir.dt.float32
    f16 = mybir.dt.float16
    op = mybir.AluOpType

    nc = bass.Bass("TRN2", target_bir_lowering=False, debug=False)

    blob = nc.dram_tensor("blob", [128, W16], f16, kind="ExternalInput")
    yout = nc.dram_tensor("y", [OUT, COLS], f32, kind="ExternalOutput")

    with tile.TileContext(nc) as tc:
        with (
            tc.tile_pool(name="const", bufs=1) as cpool,
            tc.tile_pool(name="vsb", bufs=2) as vpool,
            tc.tile_pool(name="tmp", bufs=2) as tpool,
            tc.tile_pool(name="gz", bufs=2) as gpool,
            tc.tile_pool(name="S", bufs=1) as spool,
            tc.tile_pool(name="ysb", bufs=4) as ypool_sb,
            tc.tile_pool(name="ps", bufs=1, space="PSUM") as pspool,
        ):
            b16 = cpool.tile([128, W16], f16)
            o = [0]

            def take(w):
                s = b16[:, o[0]:o[0] + w]
                o[0] += w
                return s

            u_jk = take(UW)
            ktT = [take(128) for _ in range(L)]
            ptT = [[[None] * L for _ in range(2)] for _ in range(NT)]
            ck2 = [None] * NT
            sk2 = [None] * NT
            rho2 = [None] * NT
            for m in range(NT):
                for ri in range(2):
                    for j in range(L):
                        ptT[m][ri][j] = take(128)
                ck2[m] = take(NKB)
                sk2[m] = take(NKB)
                rho2[m] = take(NKB)
            qtT = [[[None] * NT for _ in range(2)] for _ in range(L)]
            for j in range(L):
                for ri in range(2):
                    for m in range(NT):
                        qtT[j][ri][m] = take(128)
            assert o[0] == W16

            # DMA split over the two HWDGE rings:
            #   sync:   [u | K]  then [Qt]
            #   scalar: [Pt_m | tab_m | rho_m]  x 4
            a = UW + KW
            nc.sync.dma_start(b16[:, 0:a], blob[:, 0:a])
            nc.sync.dma_start(b16[:, W16 - QW:W16], blob[:, W16 - QW:W16])
            for m in range(NT):
                lo, hi = a + m * MW, a + (m + 1) * MW
                nc.scalar.dma_start(b16[:, lo:hi], blob[:, lo:hi])

            Sr_t = [None] * NT
            Si_t = [None] * NT
            for m in range(NT):
                # phase 1: vt = sum_j Pt_j u_j  (complex, PSUM)
                v_sb = [None, None]
                for ri in range(2):
                    vt = pspool.tile([128, NKB], f32, tag=f"vt{ri}", bufs=1,
                                     name=f"vt{ri}")
                    for j in range(L):
                        nc.tensor.matmul(
                            vt[:], ptT[m][ri][j], u_jk[:, j * NKB:(j + 1) * NKB],
                            start=(j == 0), stop=(j == L - 1),
                        )
                    v_sb[ri] = vpool.tile([128, NKB], f16, tag=f"v{ri}",
                                          name=f"v{ri}")
                    nc.scalar.copy(v_sb[ri][:], vt[:])
                vr, vi = v_sb
                # phase 2: modulate  g = e^{-i phi (k+1)} vt
                # (DVE: real part; GpSimd: imag part)
                t1 = tpool.tile([128, NKB], f16, tag="t1")
                t2 = tpool.tile([128, NKB], f16, tag="t2")
                nc.vector.tensor_tensor(t1[:], ck2[m], vr[:], op=op.mult)
                nc.vector.tensor_tensor(t2[:], sk2[m], vi[:], op=op.mult)
                gr = gpool.tile([128, NKB], f16, tag="gr")
                nc.vector.tensor_tensor(gr[:], t1[:], t2[:], op=op.add)
                t3 = tpool.tile([128, NKB], f16, tag="t3")
                t4 = tpool.tile([128, NKB], f16, tag="t4")
                nc.gpsimd.tensor_tensor(t3[:], ck2[m], vi[:], op=op.mult)
                nc.gpsimd.tensor_tensor(t4[:], sk2[m], vr[:], op=op.mult)
                gi = gpool.tile([128, NKB], f16, tag="gi")
                nc.gpsimd.tensor_tensor(gi[:], t3[:], t4[:], op=op.subtract)
                # scan along k; rho2 has col NK zeroed to reset state at the
                # second batch element (fp32 state, fp16 IO)
                zr = gpool.tile([128, NKB], f16, tag="zr")
                zi = gpool.tile([128, NKB], f16, tag="zi")
                nc.vector.tensor_tensor_scan(
                    zr[:], rho2[m], gr[:], 0.0, op0=op.mult, op1=op.add
                )
                nc.vector.tensor_tensor_scan(
                    zi[:], rho2[m], gi[:], 0.0, op0=op.mult, op1=op.add
                )
                # demodulate S = e^{i phi (k+1)} z, written shifted one chunk:
                # S_shift[:, b*NK + k] = S_{k-1} (= beta_k), col k=0 zeroed
                t5 = tpool.tile([128, NKB], f16, tag="t5")
                t6 = tpool.tile([128, NKB], f16, tag="t6")
                nc.vector.tensor_tensor(t5[:], ck2[m], zr[:], op=op.mult)
                nc.vector.tensor_tensor(t6[:], sk2[m], zi[:], op=op.mult)
                t7 = tpool.tile([128, NKB], f16, tag="t7")
                t8 = tpool.tile([128, NKB], f16, tag="t8")
                nc.vector.tensor_tensor(t7[:], sk2[m], zr[:], op=op.mult)
                nc.vector.tensor_tensor(t8[:], ck2[m], zi[:], op=op.mult)
                Sr = spool.tile([128, NKB], f16, tag=f"Sr{m}")
                Si = spool.tile([128, NKB], f16, tag=f"Si{m}")
                nc.vector.memset(Sr[:, 0:1], 0.0)
                nc.vector.memset(Sr[:, NK:NK + 1], 0.0)
                nc.gpsimd.memset(Si[:, 0:1], 0.0)
                nc.gpsimd.memset(Si[:, NK:NK + 1], 0.0)
                for b in range(BLOCAL):
                    a0 = b * NK
                    nc.vector.tensor_tensor(
                        Sr[:, a0 + 1:a0 + NK], t5[:, a0:a0 + NK - 1],
                        t6[:, a0:a0 + NK - 1], op=op.subtract,
                    )
                    nc.gpsimd.tensor_tensor(
                        Si[:, a0 + 1:a0 + NK], t7[:, a0:a0 + NK - 1],
                        t8[:, a0:a0 + NK - 1], op=op.add,
                    )
                Sr_t[m], Si_t[m] = Sr, Si

            # phase 3, two waves (j0..5, j6..7): conv first (only needs u/K),
            # then boundary matmuls m-outer so S(m) is consumed in completion
            # order; stop on the last accumulation (m=3, ri=1).
            for js in (range(0, 6), range(6, L)):
                yps = {}
                for j in js:
                    yps[j] = pspool.tile([128, NKB], f32, tag="y", bufs=6,
                                         name=f"y{j}")
                    for d in range(j + 1):
                        nc.tensor.matmul(
                            yps[j][:], ktT[d],
                            u_jk[:, (j - d) * NKB:(j - d + 1) * NKB],
                            start=(d == 0), stop=False, skip_group_check=True,
                        )
                for m in range(NT):
                    for j in js:
                        nc.tensor.matmul(
                            yps[j][:], qtT[j][0][m], Sr_t[m][:], start=False,
                            stop=False, skip_group_check=True,
                        )
                        nc.tensor.matmul(
                            yps[j][:], qtT[j][1][m], Si_t[m][:], start=False,
                            stop=(m == NT - 1), skip_group_check=True,
                        )
                for j in js:
                    ysb = ypool_sb.tile([128, NKB], f32, tag="ysb")
                    nc.scalar.copy(ysb[:], yps[j][:])
                    nc.gpsimd.dma_start(yout[:, j * NKB:(j + 1) * NKB], ysb[:])

    _legalize_multi_waits(nc)
    return nc


def _legalize_multi_waits(nc):
    """This walrus build accepts a single sync wait per instruction; split
    any multi-wait instruction into same-engine single-wait NoOps + the
    original carrying the last wait (program order chains them)."""
    import bass_rust
    from concourse import mybir

    uid = [0]
    for fn in nc.m.functions:
        for bb in fn.blocks:
            insts = bb.instructions
            new = []
            changed = False
            for inst in insts:
                si = inst.sync_info
                if si is not None and len(si.on_wait) > 1:
                    waits = list(si.on_wait)
                    for w in waits[:-1]:
                        uid[0] += 1
                        new.append(mybir.InstNoOp(
                            name=f"mwsplit-{uid[0]}",
                            engine=inst.engine,
                            ins=[], outs=[],
                            sync_info=bass_rust.SyncInfo(on_wait=[w], on_update=[]),
                        ))
                    inst.sync_info = bass_rust.SyncInfo(
                        on_wait=[waits[-1]], on_update=list(si.on_update)
                    )
                    changed = True
                new.append(inst)
            if changed:
                bb.instructions = new


def _host_prep(A_re, A_im, B_re, B_im, C_re, C_im, D_w):
    """fp64 eigendecomposition + chunked-formulation weight/table layouts.
    Returns shared fp16 tail of the blob: [128, W16 - UW]."""
    A = A_re.astype(np.float64) + 1j * A_im.astype(np.float64)
    w, V = np.linalg.eig(A)
    Vinv = np.linalg.inv(V)
    Bt = Vinv @ (B_re.astype(np.float64) + 1j * B_im.astype(np.float64))
    Ct = (C_re.astype(np.float64) + 1j * C_im.astype(np.float64)) @ V

    Pt = np.stack([(w ** (L - 1 - j))[:, None] * Bt for j in range(L)])  # [L,N,IN]
    Qt = np.stack([Ct * (w ** (j + 1))[None, :] for j in range(L)])      # [L,OUT,N]
    K = np.empty((L, OUT, IN))
    Ad = np.eye(N, dtype=complex)
    Bc = B_re.astype(np.float64) + 1j * B_im.astype(np.float64)
    Cc = C_re.astype(np.float64) + 1j * C_im.astype(np.float64)
    for d in range(L):
        K[d] = (Cc @ Ad @ Bc).real
        Ad = A @ Ad
    K[0] += D_w.astype(np.float64)

    wL = w ** L
    rhoL = np.abs(wL)
    phi = np.angle(wL)
    kk = np.arange(NK)
    cosk = np.cos(np.outer(phi, kk + 1))  # [N, NK]
    sink = np.sin(np.outer(phi, kk + 1))

    parts = []
    for d in range(L):
        parts.append(np.ascontiguousarray(K[d].T))  # [IN, OUT]
    for m in range(NT):
        sl = slice(m * 128, (m + 1) * 128)
        for Pp in (Pt.real, Pt.imag):
            for j in range(L):
                parts.append(np.ascontiguousarray(Pp[j].T[:, sl]))  # [IN, 128]
        parts.append(np.tile(cosk[sl], (1, BLOCAL)))  # [128, NKB]
        parts.append(np.tile(sink[sl], (1, BLOCAL)))
        rb = np.broadcast_to(rhoL[sl][:, None], (128, NKB)).copy()
        rb[:, NK] = 0.0  # reset scan state at second batch element
        parts.append(rb)
    for j in range(L):
        for Qp in (Qt[j].real, -Qt[j].imag):
            QT = np.ascontiguousarray(Qp.T)  # [N, OUT]
            for m in range(NT):
                parts.append(QT[m * 128:(m + 1) * 128])
    shared = np.concatenate(parts, axis=1).astype(np.float16)
    assert shared.shape == (128, W16 - UW)
    return shared


def _ensure_axon_hooks():
    """Provide antenv.axon_hooks if the image lacks it (needed only for
    trace=True NTFF profiling; run path works without)."""
    import types
    try:
        from antenv import axon_hooks  # noqa: F401
        return
    except ImportError:
        pass
    try:
        import antenv
        mod = types.ModuleType("antenv.axon_hooks")
        _hook = [None]
        mod.set_axon_ntff_profile_hook = lambda h: _hook.__setitem__(0, h)
        mod.get_axon_ntff_profile_hook = lambda: _hook[0]
        sys.modules["antenv.axon_hooks"] = mod
        antenv.axon_hooks = mod
        if "/root/.axon_site" not in sys.path:
            sys.path.insert(0, "/root/.axon_site")
        from trn_agent_boot.trn_boot import _ntff_profile_via_ctypes
        h = _ntff_profile_via_ctypes("/opt/axon/libaxon_pjrt.so")
        if h is not None:
            mod.set_axon_ntff_profile_hook(h)
    except Exception:
        pass


def kernel(u, A_re, A_im, B_re, B_im, C_re, C_im, D_w, output_bias):
    global LAST_RESULT, _NC_CACHE
    from concourse import bass_utils

    _ensure_axon_hooks()

    u = np.asarray(u, dtype=np.float32)
    shared = _host_prep(
        np.asarray(A_re), np.asarray(A_im), np.asarray(B_re), np.asarray(B_im),
        np.asarray(C_re), np.asarray(C_im), np.asarray(D_w)
    )

    if _NC_CACHE is None:
        _NC_CACHE = _build_nc()
    nc = _NC_CACHE

    in_maps = []
    for c in range(NCORES):
        up = u[BLOCAL * c:BLOCAL * (c + 1)]           # [2, T, IN]
        uc = up.reshape(BLOCAL, NK, L, IN)            # t = k*L + j
        u_jk = np.ascontiguousarray(
            uc.transpose(3, 2, 0, 1).reshape(IN, L * NKB)
        ).astype(np.float16)                          # col = j*NKB + b*NK + k
        in_maps.append({"blob": np.concatenate([u_jk, shared], axis=1)})

    res = bass_utils.run_bass_kernel_spmd(nc, in_maps, core_ids=list(range(NCORES)))
    LAST_RESULT = res

    y = np.empty((BATCH, T, OUT), dtype=np.float32)
    for c in range(NCORES):
        yd = res.results[c]["y"]                      # [OUT, L*NKB]
        y[BLOCAL * c:BLOCAL * (c + 1)] = (
            yd.reshape(OUT, L, BLOCAL, NK).transpose(2, 3, 1, 0)
            .reshape(BLOCAL, T, OUT)
        )
    y += np.asarray(output_bias, dtype=np.float32)
    return y
